# revision 10
# baseline (speedup 1.0000x reference)
"""GAT 2-layer (PyG GATConv x2 + BN + ReLU) on 8 Trainium2 NeuronCores.

Strategy: destination-sharded edge-parallel. Edges (with self-loops) are
sorted by destination on the host; each core owns a contiguous range of
destination nodes. Per 64-node destination window, edges are chunked into
groups of 128; per chunk the core gathers the (BN-prescaled) source
features via indirect DMA, computes per-edge attention scalars, and
accumulates both the weighted message sum and the softmax denominator
into PSUM with a single one-hot matmul per chunk. The dst-side attention
term (ad) is expanded edge-wise with a host-baked transposed one-hot
matmul. Layer-2 node features are exchanged with an AllGather.
"""
import numpy as np

N = 50000
NP = 50176            # padded to 8 cores * 49 tiles * 128
N_CORES = 8
PER = NP // N_CORES   # 6272 nodes per core
TILES = PER // 128    # 49
WIN = 64              # dst window (psum col-group)
NWIN = NP // WIN      # global windows
E_IN = 800000
IN_DIM = 256
HID = 128
HEADS = 4
DH = 32
OUT_DIM = 2
NEG_SLOPE = 0.2
BN_EPS = 1e-5

_CACHE = {}


def _split_excess_waits(nc, max_waits=1):
    import concourse.mybir as mybir
    n_split = 0
    for f in nc.m.functions:
        for bb in f.blocks:
            new_insts = []
            for inst in bb.instructions:
                si = inst.sync_info
                waits = list(si.on_wait) if si and si.on_wait else []
                if len(waits) > max_waits:
                    overflow = waits[:-max_waits]
                    for i in range(0, len(overflow), max_waits):
                        chunk = overflow[i: i + max_waits]
                        nop = mybir.InstNoOp(
                            name=f"{inst.name}-wsplit{i}",
                            engine=inst.engine,
                            sync_info=mybir.SyncInfo(on_wait=chunk, on_update=[]),
                        )
                        new_insts.append(nop)
                        n_split += 1
                    si.on_wait = waits[-max_waits:]
                new_insts.append(inst)
            bb.instructions[:] = new_insts
    return n_split


def _build_nc(K_w):
    import concourse.bass as bass
    import concourse.mybir as mybir
    from concourse.tile import TileContext
    from concourse.masks import make_identity

    f32 = mybir.dt.float32
    bf16 = mybir.dt.bfloat16
    i32 = mybir.dt.int32
    AF = mybir.ActivationFunctionType
    ALU = mybir.AluOpType

    CH = TILES * 2 * K_w       # chunks per core
    CW = 2 * K_w               # chunks per tile

    nc = bass.Bass(num_swdge_queues=4)

    # ---- per-core inputs ----
    xTs = nc.declare_dram_parameter("xTs", [IN_DIM, PER], f32, isOutput=False)
    W1 = nc.declare_dram_parameter("W1", [IN_DIM, HID], f32, isOutput=False)
    asrc = nc.declare_dram_parameter("asrc", [1, HID], f32, isOutput=False)
    adst = nc.declare_dram_parameter("adst", [1, HID], f32, isOutput=False)
    b1 = nc.declare_dram_parameter("b1", [1, HID], f32, isOutput=False)
    bng = nc.declare_dram_parameter("bng", [1, HID], f32, isOutput=False)
    bnb = nc.declare_dram_parameter("bnb", [1, HID], f32, isOutput=False)
    bnm = nc.declare_dram_parameter("bnm", [1, HID], f32, isOutput=False)
    bnv = nc.declare_dram_parameter("bnv", [1, HID], f32, isOutput=False)
    W2d = nc.declare_dram_parameter("W2", [HID, OUT_DIM], f32, isOutput=False)
    a2s = nc.declare_dram_parameter("a2s", [1, OUT_DIM], f32, isOutput=False)
    a2d = nc.declare_dram_parameter("a2d", [1, OUT_DIM], f32, isOutput=False)
    b2 = nc.declare_dram_parameter("b2", [1, OUT_DIM], f32, isOutput=False)
    gidx = nc.declare_dram_parameter("gidx", [128, CH], i32, isOutput=False)
    dstloc = nc.declare_dram_parameter("dstloc", [128, CH], f32, isOutput=False)
    ohT = nc.declare_dram_parameter("ohT", [WIN, CH * 128], f32, isOutput=False)
    iota = nc.declare_dram_parameter("iota", [128, WIN], f32, isOutput=False)
    out_ext = nc.declare_dram_parameter("out", [PER, OUT_DIM], f32, isOutput=True)
    dbg1 = nc.declare_dram_parameter("dbg1", [PER, 4], f32, isOutput=True)
    dbg2 = nc.declare_dram_parameter("dbg2", [128, 8], f32, isOutput=True)
    dbg3 = nc.declare_dram_parameter("dbg3", [PER, HID], f32, isOutput=True)
    dbg4 = nc.declare_dram_parameter("dbg4", [128, 264], f32, isOutput=True)

    # ---- internal DRAM ----
    F1slice = nc.dram_tensor("F1slice", [PER, 132], f32)
    F1full = nc.dram_tensor("F1full", [NP, 132], f32, addr_space="Shared")
    ADT1 = nc.dram_tensor("ADT1", [PER, 4], f32)
    F2slice = nc.dram_tensor("F2slice", [PER, 4], f32)
    F2full = nc.dram_tensor("F2full", [NP, 4], f32, addr_space="Shared")

    QN = ["qPoolDynamic", "qPoolDynamic1", "qPoolDynamic2", "qPoolDynamic3"]

    def gather(eng, out_ap, table_ap, idx_ap, q):
        inst = eng.indirect_dma_start(
            out=out_ap, out_offset=None, in_=table_ap,
            in_offset=bass.IndirectOffsetOnAxis(ap=idx_ap, axis=0),
        )
        inst.ins.queue = QN[q % 4]
        return inst

    with TileContext(nc) as tc:
        with (
            tc.tile_pool(name="const", bufs=1) as cp,
            tc.tile_pool(name="psA", bufs=2, space="PSUM") as psA,
            tc.tile_pool(name="psB", bufs=2, space="PSUM") as psB,
            tc.tile_pool(name="psC", bufs=2, space="PSUM") as psC,
            tc.tile_pool(name="work", bufs=3) as wp,
            tc.tile_pool(name="gat", bufs=3) as gp,
            tc.tile_pool(name="oht", bufs=2) as ohp,
            tc.tile_pool(name="sc", bufs=3) as scp,
        ):
            # ================= P0: params & folded constants =================
            ident = cp.tile([128, 128], f32)
            make_identity(nc, ident[:])
            ones1 = cp.tile([1, 128], f32)
            nc.gpsimd.memset(ones1[:], 1.0)

            prm = cp.tile([1, 9 * HID], f32, tag="prm")
            for i, srcp in enumerate([asrc, adst, b1, bng, bnb, bnm, bnv]):
                nc.sync.dma_start(out=prm[:, i * HID:(i + 1) * HID], in_=srcp[:])
            # s' = gamma / sqrt(var+eps); tshift = (b1-mean)*s' + beta
            sprime = cp.tile([1, HID], f32)
            epst = cp.tile([1, 1], f32)
            nc.gpsimd.memset(epst[:], BN_EPS)
            nc.scalar.activation(sprime[:], prm[:, 6 * HID:7 * HID], AF.Sqrt, bias=epst[:])
            nc.vector.reciprocal(sprime[:], sprime[:])
            nc.vector.tensor_tensor(out=sprime[:], in0=sprime[:], in1=prm[:, 3 * HID:4 * HID], op=ALU.mult)
            rsp = cp.tile([1, HID], f32)
            nc.vector.reciprocal(rsp[:], sprime[:])
            tsh = cp.tile([1, HID], f32)
            nc.vector.tensor_tensor(out=tsh[:], in0=prm[:, 2 * HID:3 * HID], in1=prm[:, 5 * HID:6 * HID], op=ALU.subtract)
            nc.vector.tensor_tensor(out=tsh[:], in0=tsh[:], in1=sprime[:], op=ALU.mult)
            nc.vector.tensor_tensor(out=tsh[:], in0=tsh[:], in1=prm[:, 4 * HID:5 * HID], op=ALU.add)
            ahat_s = cp.tile([1, HID], f32)
            nc.vector.tensor_tensor(out=ahat_s[:], in0=prm[:, 0:HID], in1=rsp[:], op=ALU.mult)
            ahat_d = cp.tile([1, HID], f32)
            nc.vector.tensor_tensor(out=ahat_d[:], in0=prm[:, HID:2 * HID], in1=rsp[:], op=ALU.mult)

            # replicate rows across partitions via ones-matmul
            _repc = [0]

            def repl(row_ap, width):
                ps = psC.tile([128, width], f32, tag="misc")
                nc.tensor.matmul(ps[:], lhsT=ones1[:, :128], rhs=row_ap, start=True, stop=True)
                t = cp.tile([128, width], f32, tag=f"rep{_repc[0]}"); _repc[0] += 1
                nc.vector.tensor_copy(out=t[:], in_=ps[:])
                return t

            sp_rep = repl(sprime[:], HID)
            tsh_rep = repl(tsh[:], HID)
            as_rep = repl(ahat_s[:], HID)
            ad_rep = repl(ahat_d[:], HID)

            # W1' = W1 * s'(col)  [2 x [128,128]]
            W1p = cp.tile([128, 2 * HID], f32)
            for kh in range(2):
                nc.sync.dma_start(out=W1p[:, kh * HID:(kh + 1) * HID], in_=W1[kh * 128:(kh + 1) * 128, :])
            for kh in range(2):
                nc.vector.tensor_tensor(out=W1p[:, kh * HID:(kh + 1) * HID],
                                        in0=W1p[:, kh * HID:(kh + 1) * HID], in1=sp_rep[:], op=ALU.mult)
            # Asrc/Adst [2][128,4]: reduce_d( W1'[k, (h d)] * ahat[(h d)] )
            AsT = cp.tile([128, 8], f32)   # cols: kh*4 + h  for src
            AdT = cp.tile([128, 8], f32)
            tmp = wp.tile([128, HID], f32, tag="p0tmp")
            for kh in range(2):
                nc.vector.tensor_tensor(out=tmp[:], in0=W1p[:, kh * HID:(kh + 1) * HID], in1=as_rep[:], op=ALU.mult)
                nc.vector.tensor_reduce(out=AsT[:, kh * 4:(kh + 1) * 4],
                                        in_=tmp[:].rearrange("p (h d) -> p h d", h=4),
                                        op=ALU.add, axis=mybir.AxisListType.X)
                nc.vector.tensor_tensor(out=tmp[:], in0=W1p[:, kh * HID:(kh + 1) * HID], in1=ad_rep[:], op=ALU.mult)
                nc.vector.tensor_reduce(out=AdT[:, kh * 4:(kh + 1) * 4],
                                        in_=tmp[:].rearrange("p (h d) -> p h d", h=4),
                                        op=ALU.add, axis=mybir.AxisListType.X)

            # W2A = [W2 | A2s | A2d]  [128, 4]
            W2t = cp.tile([128, OUT_DIM], f32)
            nc.sync.dma_start(out=W2t[:], in_=W2d[:])
            W2T = cp.tile([OUT_DIM, HID], f32)
            nc.sync.dma_start(out=W2T[:], in_=W2d[:].rearrange("f o -> o f"))
            a2p = cp.tile([OUT_DIM, 2], f32)
            nc.sync.dma_start(out=a2p[:, 0:1], in_=a2s[:].rearrange("one o -> o one"))
            nc.sync.dma_start(out=a2p[:, 1:2], in_=a2d[:].rearrange("one o -> o one"))
            a2t = cp.tile([1, OUT_DIM], f32)
            nc.sync.dma_start(out=a2t[:], in_=b2[:])
            psa = psC.tile([128, 2], f32, tag="misc")
            nc.tensor.matmul(psa[:], lhsT=W2T[:], rhs=a2p[:], start=True, stop=True)
            W2A = cp.tile([128, 4], f32)
            nc.vector.tensor_copy(out=W2A[:, 0:2], in_=W2t[:])
            nc.vector.tensor_copy(out=W2A[:, 2:4], in_=psa[:])
            b2_rep = repl(a2t[:], OUT_DIM)

            # index tables
            gixt = cp.tile([128, CH], i32)
            nc.sync.dma_start(out=gixt[:], in_=gidx[:])
            dlt = cp.tile([128, CH], f32)
            nc.sync.dma_start(out=dlt[:], in_=dstloc[:])
            iot = cp.tile([128, WIN], f32)
            nc.sync.dma_start(out=iot[:], in_=iota[:])

            # ================= P1: node tables (sharded) =================
            xk = cp.tile([128, 2 * PER], f32)   # xT slices: [128, PER] per k-half
            for kh in range(2):
                nc.sync.dma_start(out=xk[:, kh * PER:(kh + 1) * PER], in_=xTs[kh * 128:(kh + 1) * 128, :])

            for t in range(TILES):
                hps = psA.tile([128, HID], f32, tag="agg")
                aps = psB.tile([128, 8], f32, tag="small")
                def lt_(kh):
                    return xk[:, kh * PER + t * 128: kh * PER + (t + 1) * 128]
                for kh in range(2):
                    nc.tensor.matmul(hps[:], lhsT=lt_(kh), rhs=W1p[:, kh * HID:(kh + 1) * HID],
                                     start=(kh == 0), stop=(kh == 1))
                for kh in range(2):
                    nc.tensor.matmul(aps[:, 0:4], lhsT=lt_(kh), rhs=AsT[:, kh * 4:(kh + 1) * 4],
                                     start=(kh == 0), stop=(kh == 1))
                for kh in range(2):
                    nc.tensor.matmul(aps[:, 4:8], lhsT=lt_(kh), rhs=AdT[:, kh * 4:(kh + 1) * 4],
                                     start=(kh == 0), stop=(kh == 1))
                f1t = wp.tile([128, 132], f32, tag="f1t")
                nc.vector.tensor_copy(out=f1t[:, 0:HID], in_=hps[:])
                nc.vector.tensor_copy(out=f1t[:, HID:HID + 4], in_=aps[:, 0:4])
                adt = wp.tile([128, 4], f32, tag="adt")
                nc.vector.tensor_copy(out=adt[:], in_=aps[:, 4:8])
                nc.sync.dma_start(out=F1slice[t * 128:(t + 1) * 128, :], in_=f1t[:])
                nc.sync.dma_start(out=ADT1[t * 128:(t + 1) * 128, :], in_=adt[:])
                nc.sync.dma_start(out=dbg1[t * 128:(t + 1) * 128, :], in_=adt[:])

            nc.gpsimd.collective_compute(
                "AllGather", mybir.AluOpType.bypass,
                ins=[F1slice[:]], outs=[F1full[:]],
                replica_groups=[list(range(N_CORES))],
            )

            # ================= P2: layer-1 edge pass =================
            def edge_pass(table_full, tcols, adt_dram, adcols, agg_cols, layer):
                """Per tile: returns nothing; finalize handled by caller via
                agg psum handed back per tile."""
                pass

            qctr = [0]

            def run_layer(layer):
                tcols = 132 if layer == 1 else 4
                adh = 4 if layer == 1 else 1     # heads
                msgw = HID if layer == 1 else OUT_DIM
                table = F1full if layer == 1 else F2full
                addram = ADT1 if layer == 1 else None

                for t in range(TILES):
                    ohtile = ohp.tile([WIN, CW * 128], f32, tag="ohT")
                    nc.sync.dma_start(out=ohtile[:], in_=ohT[:, t * CW * 128:(t + 1) * CW * 128])
                    agg = psA.tile([128, msgw + adh], f32, tag="agg")
                    for w in range(2):
                        wb = w * WIN
                        cw0 = t * CW + w * K_w     # first global chunk of window
                        # ad window values, cast to bf16 [WIN, adh]
                        adw = scp.tile([WIN, adh], f32, tag="adw")
                        if layer == 1:
                            nc.sync.dma_start(out=adw[:], in_=addram[t * 128 + wb: t * 128 + wb + WIN, :])
                        else:
                            nc.sync.dma_start(out=adw[:], in_=F2slice[t * 128 + wb: t * 128 + wb + WIN, 3:4])
                        # gathers for K_w chunks
                        gt = gp.tile([128, K_w * tcols], f32, tag=f"g{layer}")
                        for c in range(K_w):
                            gather(nc.gpsimd, gt[:, c * tcols:(c + 1) * tcols], table[:],
                                   gixt[:, cw0 + c: cw0 + c + 1], qctr[0])
                            qctr[0] += 1
                        # one-hot [128, K_w*WIN]
                        oh = wp.tile([128, K_w * WIN], f32, tag="oh")
                        nc.vector.tensor_tensor(
                            out=oh[:].rearrange("p (c w) -> p c w", c=K_w),
                            in0=iot[:].unsqueeze(1).broadcast_to([128, K_w, WIN]),
                            in1=dlt[:, cw0:cw0 + K_w].unsqueeze(2).broadcast_to([128, K_w, WIN]),
                            op=ALU.is_equal)
                        # ad matmuls -> admm psum [128, K_w*adh]
                        admm = psB.tile([128, K_w * adh], f32, tag="small")
                        for c in range(K_w):
                            nc.tensor.matmul(admm[:, c * adh:(c + 1) * adh],
                                             lhsT=ohtile[:, (w * K_w + c) * 128:(w * K_w + c + 1) * 128],
                                             rhs=adw[:], start=True, stop=True)
                        # logits = as_g + ad ; leaky; exp
                        lg = scp.tile([128, K_w * adh], f32, tag="lg")
                        as_cols = gt[:].rearrange("p (c f) -> p c f", c=K_w)[:, :, msgw:msgw + adh]
                        nc.vector.tensor_tensor(out=lg[:], in0=as_cols, in1=admm[:], op=ALU.add)
                        if layer == 1 and t == 0 and w == 0:
                            dbf = wp.tile([128, 264], f32, tag="dbf")
                            nc.vector.tensor_copy(out=dbf[:, 0:132], in_=gt[:, 0:132])
                            nc.sync.dma_start(out=dbf[:, 132:264], in_=F1full[0:128, :])
                            nc.sync.dma_start(out=dbg4[:], in_=dbf[:])
                            dbt = scp.tile([128, 8], f32, tag="dbt")
                            nc.vector.tensor_copy(out=dbt[:, 0:4], in_=admm[:, 0:4])
                            nc.vector.tensor_copy(out=dbt[:, 4:8], in_=lg[:, 0:4])
                            nc.sync.dma_start(out=dbg2[:], in_=dbt[:])
                        lm = scp.tile([128, K_w * adh], f32, tag="lm")
                        nc.vector.tensor_scalar_mul(lm[:], lg[:], NEG_SLOPE)
                        nc.vector.tensor_tensor(out=lg[:], in0=lg[:], in1=lm[:], op=ALU.max)
                        ee = scp.tile([128, K_w * adh], f32, tag="ee")
                        nc.scalar.activation(ee[:], lg[:], AF.Exp)
                        # write e into gt cols [msgw:msgw+adh]; weight msg cols
                        nc.vector.tensor_copy(
                            out=gt[:].rearrange("p (c f) -> p c f", c=K_w)[:, :, msgw:msgw + adh],
                            in_=ee[:].rearrange("p (c h) -> p c h", c=K_w))
                        nc.vector.tensor_tensor(
                            out=gt[:].rearrange("p (c f) -> p c f", c=K_w)[:, :, 0:msgw]
                                .rearrange("p c (h d) -> p c h d", h=adh),
                            in0=gt[:].rearrange("p (c f) -> p c f", c=K_w)[:, :, 0:msgw]
                                .rearrange("p c (h d) -> p c h d", h=adh),
                            in1=ee[:].rearrange("p (c h) -> p c h", c=K_w)
                                .unsqueeze(3).broadcast_to([128, K_w, adh, msgw // adh]),
                            op=ALU.mult)
                        # aggregation matmuls
                        for c in range(K_w):
                            nc.tensor.matmul(agg[wb:wb + WIN, :],
                                             lhsT=oh[:, c * WIN:(c + 1) * WIN],
                                             rhs=gt[:, c * tcols:c * tcols + msgw + adh],
                                             start=(c == 0), stop=(c == K_w - 1))
                    # -------- finalize tile --------
                    if layer == 1:
                        rec = scp.tile([128, 4], f32, tag="rec")
                        nc.vector.reciprocal(rec[:], agg[:, HID:HID + 4])
                        h2 = wp.tile([128, HID], f32, tag="h2")
                        nc.vector.tensor_tensor(
                            out=h2[:].rearrange("p (h d) -> p h d", h=4),
                            in0=agg[:, 0:HID].rearrange("p (h d) -> p h d", h=4),
                            in1=rec[:].unsqueeze(2).broadcast_to([128, 4, DH]),
                            op=ALU.mult)
                        nc.vector.tensor_tensor(out=h2[:], in0=h2[:], in1=tsh_rep[:], op=ALU.add)
                        nc.vector.tensor_scalar_max(h2[:], h2[:], 0.0)
                        nc.sync.dma_start(out=dbg3[t * 128:(t + 1) * 128, :], in_=h2[:])
                        trp = psC.tile([128, 128], f32, tag="misc")
                        nc.tensor.transpose(out=trp[:], in_=h2[:], identity=ident[:])
                        h2T = wp.tile([128, 128], f32, tag="h2T")
                        nc.vector.tensor_copy(out=h2T[:], in_=trp[:])
                        f2ps = psB.tile([128, 4], f32, tag="small")
                        nc.tensor.matmul(f2ps[:], lhsT=h2T[:], rhs=W2A[:], start=True, stop=True)
                        f2t = wp.tile([128, 4], f32, tag="f2t")
                        nc.vector.tensor_copy(out=f2t[:], in_=f2ps[:])
                        nc.sync.dma_start(out=F2slice[t * 128:(t + 1) * 128, :], in_=f2t[:])
                    else:
                        rec = scp.tile([128, 1], f32, tag="rec2")
                        nc.vector.reciprocal(rec[:], agg[:, OUT_DIM:OUT_DIM + 1])
                        o2 = wp.tile([128, OUT_DIM], f32, tag="o2")
                        nc.vector.tensor_tensor(
                            out=o2[:], in0=agg[:, 0:OUT_DIM],
                            in1=rec[:].broadcast_to([128, OUT_DIM]), op=ALU.mult)
                        nc.vector.tensor_tensor(out=o2[:], in0=o2[:], in1=b2_rep[:], op=ALU.add)
                        nc.sync.dma_start(out=out_ext[t * 128:(t + 1) * 128, :], in_=o2[:])

            run_layer(1)
            nc.gpsimd.collective_compute(
                "AllGather", mybir.AluOpType.bypass,
                ins=[F2slice[:]], outs=[F2full[:]],
                replica_groups=[list(range(N_CORES))],
            )
            run_layer(2)

    _split_excess_waits(nc)
    return nc


def _make_runner(nc):
    import time
    import jax
    from jax.sharding import Mesh, PartitionSpec
    from jax.experimental.shard_map import shard_map
    import concourse.mybir as mybir
    from concourse import bass2jax
    from concourse.bass2jax import _bass_exec_p, install_neuronx_cc_hook

    install_neuronx_cc_hook()
    partition_name = nc.partition_id_tensor.name if nc.partition_id_tensor else None
    in_names, out_names, out_avals, zero_outs = [], [], [], []
    for alloc in nc.m.functions[0].allocations:
        if not isinstance(alloc, mybir.MemoryLocationSet):
            continue
        name = alloc.memorylocations[0].name
        if alloc.kind == "ExternalInput":
            if name != partition_name:
                in_names.append(name)
        elif alloc.kind == "ExternalOutput":
            out_names.append(name)
            shape = tuple(alloc.tensor_shape)
            dtype = mybir.dt.np(alloc.dtype)
            out_avals.append(jax.core.ShapedArray(shape, dtype))
            zero_outs.append(np.zeros(shape, dtype))
    n_params = len(in_names)
    n_outs = len(out_avals)
    all_in = list(in_names) + list(out_names)
    if partition_name is not None:
        all_in.append(partition_name)
    donate = tuple(range(n_params, n_params + n_outs))

    def _body(*args):
        operands = list(args)
        if partition_name is not None:
            operands.append(bass2jax.partition_id_tensor())
        return tuple(_bass_exec_p.bind(
            *operands, out_avals=tuple(out_avals), in_names=tuple(all_in),
            out_names=tuple(out_names), lowering_input_output_aliases=(),
            sim_require_finite=False, sim_require_nnan=False, nc=nc))

    devices = jax.devices()[:N_CORES]
    mesh = Mesh(np.asarray(devices), ("core",))
    sharded = jax.jit(
        shard_map(_body, mesh=mesh,
                  in_specs=(PartitionSpec("core"),) * (n_params + n_outs),
                  out_specs=(PartitionSpec("core"),) * len(out_names),
                  check_rep=False),
        donate_argnums=donate, keep_unused=True)

    def run(in_maps):
        per_core = [[np.asarray(m[name]) for name in in_names] for m in in_maps]
        concat_in = [np.concatenate([per_core[c][i] for c in range(N_CORES)], axis=0)
                     for i in range(n_params)]
        zs = [np.zeros((N_CORES * z.shape[0], *z.shape[1:]), z.dtype) for z in zero_outs]
        out_arrs = sharded(*concat_in, *zs)
        return [
            {name: np.asarray(out_arrs[i]).reshape(N_CORES, *out_avals[i].shape)[c]
             for i, name in enumerate(out_names)}
            for c in range(N_CORES)
        ]

    return run


def _preprocess(edge_index):
    src = np.concatenate([np.asarray(edge_index[0]), np.arange(N, dtype=np.int64)]).astype(np.int64)
    dst = np.concatenate([np.asarray(edge_index[1]), np.arange(N, dtype=np.int64)]).astype(np.int64)
    order = np.argsort(dst, kind="stable")
    src_s = src[order].astype(np.int32)
    dst_s = dst[order].astype(np.int32)
    gw = dst_s // WIN
    counts = np.bincount(gw, minlength=NWIN)
    K_w = int(np.ceil(counts.max() / 128))
    CH = TILES * 2 * K_w
    starts = np.zeros(NWIN + 1, np.int64)
    np.cumsum(counts, out=starts[1:])

    gidx = np.zeros((N_CORES, 128, CH), np.int32)
    dstloc = np.full((N_CORES, 128, CH), -1.0, np.float32)
    for wgl in range(NWIN):
        core = wgl // (PER // WIN)
        wloc = wgl % (PER // WIN)
        c0 = wloc * K_w
        s, e = starts[wgl], starts[wgl + 1]
        n = e - s
        if n == 0:
            continue
        j = np.arange(n)
        p = j % 128
        cc = c0 + j // 128
        gidx[core][p, cc] = src_s[s:e]
        dstloc[core][p, cc] = (dst_s[s:e] - wgl * WIN).astype(np.float32)

    ohT = np.zeros((N_CORES, WIN, CH * 128), np.float32)
    wr = np.arange(WIN, dtype=np.float32)
    for core in range(N_CORES):
        dl = dstloc[core]                       # [128, CH]
        oh = (dl.T[None, :, :] == wr[:, None, None])   # [WIN, CH, 128]
        ohT[core] = oh.reshape(WIN, CH * 128).astype(np.float32)
    return K_w, gidx, dstloc, ohT


def kernel(x, edge_index, W1, att_src1, att_dst1, b1,
           bn_gamma, bn_beta, bn_mean, bn_var,
           W2, att_src2, att_dst2, b2):
    x = np.asarray(x, np.float32)
    K_w, gidx, dstloc, ohT = _preprocess(edge_index)

    key = ("nc", K_w)
    if key not in _CACHE:
        nc = _build_nc(K_w)
        _CACHE[key] = _make_runner(nc)
    run = _CACHE[key]

    xp = np.zeros((NP, IN_DIM), np.float32)
    xp[:N] = x
    iota = np.broadcast_to(np.arange(WIN, dtype=np.float32), (128, WIN)).copy()

    in_maps = []
    for c in range(N_CORES):
        xTs = np.ascontiguousarray(xp[c * PER:(c + 1) * PER].T)
        in_maps.append({
            "xTs": xTs,
            "W1": np.asarray(W1, np.float32),
            "asrc": np.asarray(att_src1, np.float32).reshape(1, HID),
            "adst": np.asarray(att_dst1, np.float32).reshape(1, HID),
            "b1": np.asarray(b1, np.float32).reshape(1, HID),
            "bng": np.asarray(bn_gamma, np.float32).reshape(1, HID),
            "bnb": np.asarray(bn_beta, np.float32).reshape(1, HID),
            "bnm": np.asarray(bn_mean, np.float32).reshape(1, HID),
            "bnv": np.asarray(bn_var, np.float32).reshape(1, HID),
            "W2": np.asarray(W2, np.float32),
            "a2s": np.asarray(att_src2, np.float32).reshape(1, OUT_DIM),
            "a2d": np.asarray(att_dst2, np.float32).reshape(1, OUT_DIM),
            "b2": np.asarray(b2, np.float32).reshape(1, OUT_DIM),
            "gidx": gidx[c],
            "dstloc": dstloc[c],
            "ohT": ohT[c],
            "iota": iota,
        })

    results = run(in_maps)
    kernel._last_results = results
    out = np.concatenate([results[c]["out"] for c in range(N_CORES)], axis=0)
    return out[:N].astype(np.float32)
